# revision 4
# baseline (speedup 1.0000x reference)
"""Trainium2 Bass kernel for the pairwise-distance softmax hinge-embedding loss.

Reference math (n = m = 8192, d = 128):
    logits[i,j] = -||f1_i - f2_j||^2
    probs = softmax(logits, axis=1)
    loss  = mean( where(simi==1, probs, max(0, 1 - probs)) )

Because probs in [0,1], max(0, 1-p) == 1-p, so with s = simi in {+1,-1}:
    loss * n^2 = count(s == -1) + sum_ij s_ij * p_ij
               = (n^2 - R)/2 + sum_i t_i / Z_i
where, using the softmax shift-invariance to drop the per-row sq1_i term,
    e_ij = exp(2*<f1_i, f2_j> - ||f2_j||^2)     (verified to stay in fp32 range)
    Z_i  = sum_j e_ij,   t_i = sum_j s_ij e_ij,  R = sum_ij s_ij

Sharding: rows of feat1/simi are split across the 8 cores (1024 rows each);
feat2 is replicated.  Each core computes q_i = t_i/Z_i and per-row sums of s
for its row block; the host adds the tiny per-row outputs together.

Per-core engine plan (row-major layout, i on partitions, j on free dim):
  - TensorE:  c-matmuls (bf16) + K=1 "inject" matmuls adding -||f2_j||^2
  - ScalarE:  exp from PSUM -> bf16 SBUF with fused free-dim accumulation (Z)
  - VectorE:  one fused tensor_tensor_reduce (s*e, accum t) + one
              tensor_scalar accumulation pass (R partials)
  - DMA:      simi tiles are cast int32 -> bf16 during the (SWDGE) DMA
"""

import sys

if "/opt/trn_rl_repo" not in sys.path:
    sys.path.insert(0, "/opt/trn_rl_repo")

import numpy as np
import ml_dtypes

N_CORES = 8
N = 8192  # rows (feat1) and cols (feat2)
D = 128
NI = N // N_CORES  # rows per core = 1024
IC = NI // 128     # i-chunks per core = 8
JT_FD = 1024       # free-dim tile width along j
JT = N // JT_FD    # j-tiles = 8

_CACHED = {}


def _build_module():
    """Build (and cache) the Bass module shared by all 8 cores."""
    if "nc" in _CACHED:
        return _CACHED["nc"]

    import concourse.bass as bass
    import concourse.bacc as bacc
    import concourse.tile as tile
    from concourse import mybir

    f32 = mybir.dt.float32
    bf16 = mybir.dt.bfloat16
    i32 = mybir.dt.int32

    nc = bacc.Bacc(
        "TRN2",
        debug=False,
        enable_asserts=False,
        target_bir_lowering=False,
        num_devices=N_CORES,
    )

    f1t = nc.dram_tensor("f1t", [D, NI], f32, kind="ExternalInput").ap()
    f2t = nc.dram_tensor("f2t", [D, N], f32, kind="ExternalInput").ap()
    s_in = nc.dram_tensor("s", [NI, N], i32, kind="ExternalInput").ap()
    q_out = nc.dram_tensor("q", [128, IC], f32, kind="ExternalOutput").ap()
    r_out = nc.dram_tensor("r", [128, IC], f32, kind="ExternalOutput").ap()

    with tile.TileContext(nc) as tc:
        with (
            tc.tile_pool(name="const", bufs=1) as const,
            tc.tile_pool(name="stage", bufs=1) as stage,
            tc.tile_pool(name="spool", bufs=8) as spool,
            tc.tile_pool(name="epool", bufs=3) as epool,
            tc.tile_pool(name="junk", bufs=2) as junk,
            tc.tile_pool(name="stats", bufs=1) as stats,
            tc.tile_pool(name="psum", bufs=3, space="PSUM") as psum,
            tc.tile_pool(name="psq", bufs=2, space="PSUM") as psq,
        ):
            # ---------------- prolog: feat tiles, sq2 row ----------------
            f2t_f32 = stage.tile([D, N], f32)
            nc.sync.dma_start(out=f2t_f32, in_=f2t)
            f1t_f32 = stage.tile([D, NI], f32)
            nc.sync.dma_start(out=f1t_f32, in_=f1t)

            f2t_b = const.tile([D, N], bf16)
            nc.vector.tensor_copy(f2t_b, f2t_f32)
            # stationary operand: 2*f1 (folds the factor 2 of the cross term)
            f1t_b = const.tile([D, NI], bf16)
            nc.vector.tensor_scalar_mul(f1t_b, f1t_f32, 2.0)

            # sq2_j = sum_d f2[d,j]^2 via ones-matmul (fp32 for precision)
            sqf = stage.tile([D, N], f32)
            nc.vector.tensor_mul(sqf, f2t_f32, f2t_f32)
            ones_col = const.tile([D, 1], f32)
            nc.vector.memset(ones_col, 1.0)
            negsq2 = const.tile([1, N], f32)
            for k in range(N // 512):
                pq = psq.tile([1, 512], f32)
                nc.tensor.matmul(
                    pq, lhsT=ones_col, rhs=sqf[:, k * 512 : (k + 1) * 512],
                    start=True, stop=True,
                )
                nc.scalar.activation(
                    out=negsq2[:, k * 512 : (k + 1) * 512], in_=pq,
                    func=mybir.ActivationFunctionType.Copy, scale=-1.0, bias=0.0,
                )
            # K=1 stationary ones row for the inject matmuls
            ones_row = const.tile([1, 128], f32)
            nc.vector.memset(ones_row, 1.0)

            # per-(ic, jt) partials
            zp = stats.tile([128, IC, JT], f32)
            tp = stats.tile([128, IC, JT], f32)
            rp = stats.tile([128, IC, JT], f32)

            # ---------------- main loop ----------------
            for ic in range(IC):
                lhs = f1t_b[:, ic * 128 : (ic + 1) * 128]
                for jt in range(JT):
                    j0 = jt * JT_FD
                    s_sb = spool.tile([128, JT_FD], bf16, tag="s")
                    nc.gpsimd.dma_start(
                        out=s_sb,
                        in_=s_in[ic * 128 : (ic + 1) * 128, j0 : j0 + JT_FD],
                    )

                    L = psum.tile([128, JT_FD], f32, tag="L")
                    for h in range(JT_FD // 512):
                        c0 = h * 512
                        nc.tensor.matmul(
                            L[:, c0 : c0 + 512],
                            lhsT=lhs,
                            rhs=f2t_b[:, j0 + c0 : j0 + c0 + 512],
                            start=True, stop=False,
                        )
                        nc.tensor.matmul(
                            L[:, c0 : c0 + 512],
                            lhsT=ones_row,
                            rhs=negsq2[:, j0 + c0 : j0 + c0 + 512],
                            start=False, stop=True,
                        )

                    e_sb = epool.tile([128, JT_FD], bf16, tag="e")
                    nc.scalar.activation(
                        out=e_sb, in_=L,
                        func=mybir.ActivationFunctionType.Exp,
                        scale=1.0, bias=0.0,
                        accum_out=zp[:, ic, jt : jt + 1],
                    )

                    se_sb = junk.tile([128, JT_FD], bf16, tag="se")
                    nc.vector.affine_mul_reduce(
                        out=se_sb, accum_out=tp[:, ic, jt : jt + 1],
                        in0=e_sb, in1=s_sb, scale=1.0, bias=0.0,
                    )
                    r_junk = junk.tile([128, JT_FD], bf16, tag="rj")
                    nc.vector.tensor_scalar(
                        out=r_junk, in0=s_sb, scalar1=1.0, scalar2=0.0,
                        op0=mybir.AluOpType.mult, op1=mybir.AluOpType.add,
                        accum_out=rp[:, ic, jt : jt + 1],
                    )

            # ---------------- epilog: q = t/Z, r row-sums ----------------
            zt = stats.tile([128, IC], f32)
            tt = stats.tile([128, IC], f32)
            rt = stats.tile([128, IC], f32)
            nc.vector.reduce_sum(zt, zp, axis=mybir.AxisListType.X)
            nc.vector.reduce_sum(tt, tp, axis=mybir.AxisListType.X)
            nc.vector.reduce_sum(rt, rp, axis=mybir.AxisListType.X)
            zinv = stats.tile([128, IC], f32)
            nc.vector.reciprocal(zinv, zt)
            qv = stats.tile([128, IC], f32)
            nc.vector.tensor_mul(qv, tt, zinv)
            nc.sync.dma_start(out=q_out, in_=qv)
            nc.sync.dma_start(out=r_out, in_=rt)

    nc.compile()
    _CACHED["nc"] = nc
    return nc


def _run(feat1, feat2, simi, trace=False, **kwargs):
    from concourse import bass_utils

    nc = _build_module()

    feat1 = np.asarray(feat1, dtype=np.float32)
    feat2 = np.asarray(feat2, dtype=np.float32)
    simi = np.asarray(simi, dtype=np.int32)

    f1t_full = np.ascontiguousarray(feat1.T)  # [D, N]
    f2t_full = np.ascontiguousarray(feat2.T)  # [D, N]

    in_maps = []
    for c in range(N_CORES):
        in_maps.append(
            {
                "f1t": np.ascontiguousarray(f1t_full[:, c * NI : (c + 1) * NI]),
                "f2t": f2t_full,
                "s": np.ascontiguousarray(simi[c * NI : (c + 1) * NI, :]),
            }
        )

    res = bass_utils.run_bass_kernel_spmd(
        nc, in_maps, core_ids=list(range(N_CORES)), trace=trace, **kwargs
    )

    total_q = 0.0
    total_r = 0.0
    for c in range(N_CORES):
        total_q += float(np.sum(res.results[c]["q"], dtype=np.float64))
        total_r += float(np.sum(res.results[c]["r"], dtype=np.float64))

    n2 = float(N) * float(N)
    loss = ((n2 - total_r) / 2.0 + total_q) / n2
    return np.float32(loss), res


def kernel(feat1: np.ndarray, feat2: np.ndarray, simi: np.ndarray) -> np.ndarray:
    loss, _ = _run(feat1, feat2, simi)
    return loss


# revision 5
# speedup vs baseline: 1.6100x; 1.6100x over previous
"""Trainium2 Bass kernel for the pairwise-distance softmax hinge-embedding loss.

Reference math (n = m = 8192, d = 128):
    logits[i,j] = -||f1_i - f2_j||^2
    probs = softmax(logits, axis=1)
    loss  = mean( where(simi==1, probs, max(0, 1 - probs)) )

Because probs in [0,1], max(0, 1-p) == 1-p, so with s = simi in {+1,-1}:
    loss * n^2 = count(s == -1) + sum_ij s_ij * p_ij
               = (n^2 - R)/2 + sum_i t_i / Z_i
where, using the softmax shift-invariance to drop the per-row sq1_i term,
    e_ij = exp(2*<f1_i, f2_j> - ||f2_j||^2)     (verified to stay in fp32 range)
    Z_i  = sum_j e_ij,   t_i = sum_j s_ij e_ij,  R = sum_ij s_ij

Sharding: rows of feat1/simi are split across the 8 cores (1024 rows each);
feat2 is replicated.  Each core computes q_i = t_i/Z_i for its row block plus
column-sum partials of s; the host adds the tiny per-row outputs together.

Per-core engine plan (row-major layout, i on partitions, j on free dim):
  - TensorE (all bf16): c-matmul, K=1 inject matmul adding -||f2_j||^2 into
    PSUM, and a column-sum matmul of s accumulated across all tiles (for R)
  - ScalarE: exp from PSUM -> bf16 SBUF with fused free-dim accumulation (Z)
  - VectorE: one fused affine_mul_reduce (s*e, accum t) per tile
  - DMA:     simi tiles are cast int32 -> bf16 during the (SWDGE) DMA
"""

import sys

if "/opt/trn_rl_repo" not in sys.path:
    sys.path.insert(0, "/opt/trn_rl_repo")

import numpy as np
import ml_dtypes

N_CORES = 8
N = 8192  # rows (feat1) and cols (feat2)
D = 128
NI = N // N_CORES  # rows per core = 1024
IC = NI // 128     # i-chunks per core = 8
JT_FD = 1024       # free-dim tile width along j
JT = N // JT_FD    # j-tiles = 8

_CACHED = {}


def _build_module():
    """Build (and cache) the Bass module shared by all 8 cores."""
    if "nc" in _CACHED:
        return _CACHED["nc"]

    import concourse.bass as bass
    import concourse.bacc as bacc
    import concourse.tile as tile
    from concourse import mybir

    f32 = mybir.dt.float32
    bf16 = mybir.dt.bfloat16
    i32 = mybir.dt.int32

    nc = bacc.Bacc(
        "TRN2",
        debug=False,
        enable_asserts=False,
        target_bir_lowering=False,
        num_devices=N_CORES,
    )

    f1t = nc.dram_tensor("f1t", [D, NI], f32, kind="ExternalInput").ap()
    f2t = nc.dram_tensor("f2t", [D, N], f32, kind="ExternalInput").ap()
    s_in = nc.dram_tensor("s", [NI, N], i32, kind="ExternalInput").ap()
    q_out = nc.dram_tensor("q", [128, IC], f32, kind="ExternalOutput").ap()
    cs_out = nc.dram_tensor("cs", [1, JT_FD], f32, kind="ExternalOutput").ap()

    with tile.TileContext(nc) as tc:
        with (
            tc.tile_pool(name="const", bufs=1) as const,
            tc.tile_pool(name="stage", bufs=1) as stage,
            tc.tile_pool(name="spool", bufs=8) as spool,
            tc.tile_pool(name="epool", bufs=3) as epool,
            tc.tile_pool(name="junk", bufs=2) as junk,
            tc.tile_pool(name="stats", bufs=1) as stats,
            tc.tile_pool(name="psum", bufs=2, space="PSUM") as psum,
            tc.tile_pool(name="pcs", bufs=1, space="PSUM") as pcs,
            tc.tile_pool(name="psq", bufs=1, space="PSUM") as psq,
        ):
            # ---------------- prolog: feat tiles, -sq2 row (all bf16) ------
            f2t_f32 = stage.tile([D, N], f32)
            nc.sync.dma_start(out=f2t_f32, in_=f2t)
            f1t_f32 = stage.tile([D, NI], f32)
            nc.sync.dma_start(out=f1t_f32, in_=f1t)

            f2t_b = const.tile([D, N], bf16)
            nc.vector.tensor_copy(f2t_b, f2t_f32)
            # stationary operand: 2*f1 (folds the factor 2 of the cross term)
            f1t_b = const.tile([D, NI], bf16)
            nc.vector.tensor_scalar_mul(f1t_b, f1t_f32, 2.0)

            # -sq2_j = -sum_d f2[d,j]^2 via (-1)-matmul over bf16 squares
            sqb = stage.tile([D, N], bf16)
            nc.vector.tensor_mul(sqb, f2t_b, f2t_b)
            neg_col = const.tile([D, 1], bf16)
            nc.vector.memset(neg_col, -1.0)
            negsq2 = const.tile([1, N], bf16)
            for k in range(N // 512):
                pq = psq.tile([1, 512], f32)
                nc.tensor.matmul(
                    pq, lhsT=neg_col, rhs=sqb[:, k * 512 : (k + 1) * 512],
                    start=True, stop=True,
                )
                nc.scalar.activation(
                    out=negsq2[:, k * 512 : (k + 1) * 512], in_=pq,
                    func=mybir.ActivationFunctionType.Copy, scale=1.0, bias=0.0,
                )
            ones_row = const.tile([1, 128], bf16)
            nc.vector.memset(ones_row, 1.0)
            ones_col = const.tile([D, 1], bf16)
            nc.vector.memset(ones_col, 1.0)

            # per-(ic, jt) partials
            zp = stats.tile([128, IC, JT], f32)
            tp = stats.tile([128, IC, JT], f32)

            # column-sum accumulator for R, accumulated across all tiles
            cs_ps = pcs.tile([1, JT_FD], f32)

            # ---------------- main loop (jt outer: negsq2 chunks pipeline) --
            n_tiles = JT * IC
            tix = 0
            for jt in range(JT):
                j0 = jt * JT_FD
                for ic in range(IC):
                    s_sb = spool.tile([128, JT_FD], bf16, tag="s")
                    nc.gpsimd.dma_start(
                        out=s_sb,
                        in_=s_in[ic * 128 : (ic + 1) * 128, j0 : j0 + JT_FD],
                    )

                    L = psum.tile([128, JT_FD], f32, tag="L")
                    lhs = f1t_b[:, ic * 128 : (ic + 1) * 128]
                    for h in range(JT_FD // 512):
                        c0 = h * 512
                        nc.tensor.matmul(
                            L[:, c0 : c0 + 512],
                            lhsT=lhs,
                            rhs=f2t_b[:, j0 + c0 : j0 + c0 + 512],
                            start=True, stop=False,
                        )
                        nc.tensor.matmul(
                            L[:, c0 : c0 + 512],
                            lhsT=ones_row,
                            rhs=negsq2[:, j0 + c0 : j0 + c0 + 512],
                            start=False, stop=True,
                        )
                        # column sums of s for R (accumulates over all tiles)
                        nc.tensor.matmul(
                            cs_ps[:, c0 : c0 + 512],
                            lhsT=ones_col,
                            rhs=s_sb[:, c0 : c0 + 512],
                            start=(tix == 0), stop=(tix == n_tiles - 1),
                            skip_group_check=True,
                        )

                    e_sb = epool.tile([128, JT_FD], bf16, tag="e")
                    nc.scalar.activation(
                        out=e_sb, in_=L,
                        func=mybir.ActivationFunctionType.Exp,
                        scale=1.0, bias=0.0,
                        accum_out=zp[:, ic, jt : jt + 1],
                    )

                    se_sb = junk.tile([128, JT_FD], bf16, tag="se")
                    nc.vector.affine_mul_reduce(
                        out=se_sb, accum_out=tp[:, ic, jt : jt + 1],
                        in0=e_sb, in1=s_sb, scale=1.0, bias=0.0,
                    )
                    tix += 1

            # ---------------- epilog: q = t/Z, column sums out -------------
            zt = stats.tile([128, IC], f32)
            tt = stats.tile([128, IC], f32)
            nc.vector.reduce_sum(zt, zp, axis=mybir.AxisListType.X)
            nc.vector.reduce_sum(tt, tp, axis=mybir.AxisListType.X)
            zinv = stats.tile([128, IC], f32)
            nc.vector.reciprocal(zinv, zt)
            qv = stats.tile([128, IC], f32)
            nc.vector.tensor_mul(qv, tt, zinv)
            nc.sync.dma_start(out=q_out, in_=qv)
            cs_sb = stats.tile([1, JT_FD], f32)
            nc.vector.tensor_copy(cs_sb, cs_ps)
            nc.sync.dma_start(out=cs_out, in_=cs_sb)

    nc.compile()
    _CACHED["nc"] = nc
    return nc


def _run(feat1, feat2, simi, trace=False, **kwargs):
    from concourse import bass_utils

    nc = _build_module()

    feat1 = np.asarray(feat1, dtype=np.float32)
    feat2 = np.asarray(feat2, dtype=np.float32)
    simi = np.asarray(simi, dtype=np.int32)

    f1t_full = np.ascontiguousarray(feat1.T)  # [D, N]
    f2t_full = np.ascontiguousarray(feat2.T)  # [D, N]

    in_maps = []
    for c in range(N_CORES):
        in_maps.append(
            {
                "f1t": np.ascontiguousarray(f1t_full[:, c * NI : (c + 1) * NI]),
                "f2t": f2t_full,
                "s": np.ascontiguousarray(simi[c * NI : (c + 1) * NI, :]),
            }
        )

    res = bass_utils.run_bass_kernel_spmd(
        nc, in_maps, core_ids=list(range(N_CORES)), trace=trace, **kwargs
    )

    total_q = 0.0
    total_r = 0.0
    for c in range(N_CORES):
        total_q += float(np.sum(res.results[c]["q"], dtype=np.float64))
        total_r += float(np.sum(res.results[c]["cs"], dtype=np.float64))

    n2 = float(N) * float(N)
    loss = ((n2 - total_r) / 2.0 + total_q) / n2
    return np.float32(loss), res


def kernel(feat1: np.ndarray, feat2: np.ndarray, simi: np.ndarray) -> np.ndarray:
    loss, _ = _run(feat1, feat2, simi)
    return loss


# revision 8
# speedup vs baseline: 1.7531x; 1.0889x over previous
"""Trainium2 Bass kernel for the pairwise-distance softmax hinge-embedding loss.

Reference math (n = m = 8192, d = 128):
    logits[i,j] = -||f1_i - f2_j||^2
    probs = softmax(logits, axis=1)
    loss  = mean( where(simi==1, probs, max(0, 1 - probs)) )

Because probs in [0,1], max(0, 1-p) == 1-p, so with s = simi in {+1,-1}:
    loss * n^2 = count(s == -1) + sum_ij s_ij * p_ij
               = (n^2 - R)/2 + sum_i t_i / Z_i
where, using the softmax shift-invariance to drop the per-row sq1_i term,
    e_ij = exp(2*(<f1_i, f2_j> - ||f2_j||^2/2))   (verified to stay in fp32 range)
    Z_i  = sum_j e_ij,   t_i = sum_j s_ij e_ij,  R = sum_ij s_ij

Sharding: rows of feat1/simi are split across the 8 cores (1024 rows each);
feat2 is replicated.  Each core computes q_i = t_i/Z_i for its row block plus
column-sum partials of s; the host adds the tiny per-row outputs together.

Per-core engine plan (row-major layout, i on partitions, j on free dim):
  - TensorE (all bf16): c-matmul, K=1 inject matmul adding -||f2_j||^2/2 into
    PSUM, and a column-sum matmul of s accumulated across all tiles (for R).
    A dummy-matmul warmup burst keeps the PE HAM clock at 2.4 GHz through the
    prolog; matmuls are pair-batched to amortize LDWEIGHTS.
  - ScalarE: exp(2*psum) from PSUM -> bf16 SBUF with fused accumulation (Z)
  - VectorE: one fused affine_mul_reduce (s*e, accum t) per tile
  - DMA:     feat and simi tiles are cast to bf16 during the (SWDGE) DMA
"""

import sys

if "/opt/trn_rl_repo" not in sys.path:
    sys.path.insert(0, "/opt/trn_rl_repo")

import numpy as np
import ml_dtypes

N_CORES = 8
N = 8192  # rows (feat1) and cols (feat2)
D = 128
NI = N // N_CORES  # rows per core = 1024
IC = NI // 128     # i-chunks per core = 8
JT_FD = 1024       # free-dim compute tile width along j
JT = N // JT_FD    # compute j-tiles per i-chunk = 8
SD_FD = 2048       # free-dim width of one simi DMA (1 MiB transfers)
N_WARMUP = 72      # dummy matmuls covering the prolog

_CACHED = {}


def _build_module():
    """Build (and cache) the Bass module shared by all 8 cores."""
    if "nc" in _CACHED:
        return _CACHED["nc"]

    import concourse.bass as bass
    import concourse.bacc as bacc
    import concourse.tile as tile
    from concourse import mybir

    f32 = mybir.dt.float32
    bf16 = mybir.dt.bfloat16
    i32 = mybir.dt.int32

    nc = bacc.Bacc(
        "TRN2",
        debug=False,
        enable_asserts=False,
        target_bir_lowering=False,
        num_devices=N_CORES,
    )

    f1t = nc.dram_tensor("f1t", [D, NI], f32, kind="ExternalInput").ap()
    f2t = nc.dram_tensor("f2t", [D, N], f32, kind="ExternalInput").ap()
    s_in = nc.dram_tensor("s", [NI, N], i32, kind="ExternalInput").ap()
    q_out = nc.dram_tensor("q", [128, IC], f32, kind="ExternalOutput").ap()
    cs_out = nc.dram_tensor("cs", [1, JT_FD], f32, kind="ExternalOutput").ap()

    with tile.TileContext(nc) as tc:
        with (
            tc.tile_pool(name="const", bufs=1) as const,
            tc.tile_pool(name="stage", bufs=1) as stage,
            tc.tile_pool(name="spool", bufs=4) as spool,
            tc.tile_pool(name="epool", bufs=3) as epool,
            tc.tile_pool(name="junk", bufs=2) as junk,
            tc.tile_pool(name="stats", bufs=1) as stats,
            tc.tile_pool(name="psum", bufs=2, space="PSUM") as psum,
            tc.tile_pool(name="pcs", bufs=1, space="PSUM") as pcs,
            tc.tile_pool(name="psq", bufs=1, space="PSUM") as psq,
            tc.tile_pool(name="pwarm", bufs=1, space="PSUM") as pwarm,
        ):
            # ------------- PE warmup burst (keeps HAM at 2.4 GHz) ----------
            wsrc = const.tile([D, 512], bf16)
            nc.vector.memset(wsrc, 1.0)
            wp = pwarm.tile([1, 512], f32)
            for w in range(N_WARMUP):
                nc.tensor.matmul(
                    wp, lhsT=wsrc[:, 0:1], rhs=wsrc,
                    start=True, stop=True, skip_group_check=True,
                )
            # keep the warmup chain alive past DCE
            wkeep = stats.tile([1, 1], f32)
            nc.vector.tensor_copy(wkeep, wp[:, 0:1])

            # ------------- prolog: feat tiles (DMA-cast), -sq2/2 row -------
            f2t_b = const.tile([D, N], bf16)
            nc.gpsimd.dma_start(out=f2t_b, in_=f2t)
            f1t_b = const.tile([D, NI], bf16)
            nc.gpsimd.dma_start(out=f1t_b, in_=f1t)

            # -sq2_j/2 = -0.5 * sum_d f2[d,j]^2 via (-0.5)-matmul of squares
            sqb = stage.tile([D, N], bf16)
            nc.vector.tensor_mul(sqb, f2t_b, f2t_b)
            neghalf_col = const.tile([D, 1], bf16)
            nc.vector.memset(neghalf_col, -0.5)
            negsq2 = const.tile([1, N], bf16)
            for k in range(N // 512):
                pq = psq.tile([1, 512], f32)
                nc.tensor.matmul(
                    pq, lhsT=neghalf_col, rhs=sqb[:, k * 512 : (k + 1) * 512],
                    start=True, stop=True,
                )
                nc.scalar.activation(
                    out=negsq2[:, k * 512 : (k + 1) * 512], in_=pq,
                    func=mybir.ActivationFunctionType.Copy, scale=1.0, bias=0.0,
                )
            ones_row = const.tile([1, 128], bf16)
            nc.vector.memset(ones_row, 1.0)
            ones_col = const.tile([D, 1], bf16)
            nc.vector.memset(ones_col, 1.0)

            # per-(ic, jt) partials
            zp = stats.tile([128, IC, JT], f32)
            tp = stats.tile([128, IC, JT], f32)

            # column-sum accumulator for R, accumulated across all tiles
            cs_ps = pcs.tile([1, JT_FD], f32)

            # ------------- main loop: ic outer, jt pair-batched ------------
            n_tiles = IC * JT
            tix = 0
            for ic in range(IC):
                lhs = f1t_b[:, ic * 128 : (ic + 1) * 128]
                s_chunks = {}
                for jd in range(N // SD_FD):
                    sc = spool.tile([128, SD_FD], bf16, tag="s")
                    nc.gpsimd.dma_start(
                        out=sc,
                        in_=s_in[
                            ic * 128 : (ic + 1) * 128,
                            jd * SD_FD : (jd + 1) * SD_FD,
                        ],
                    )
                    s_chunks[jd] = sc

                for jp in range(JT // 2):  # pairs of compute tiles
                    pair = []
                    for h in range(2):
                        jt = jp * 2 + h
                        j0 = jt * JT_FD
                        sc = s_chunks[j0 // SD_FD]
                        s_sb = sc[:, (j0 % SD_FD) : (j0 % SD_FD) + JT_FD]
                        L = psum.tile([128, JT_FD], f32, tag="L")
                        pair.append((jt, j0, s_sb, L))

                    # c-matmuls (shared f1 weights)
                    for jt, j0, s_sb, L in pair:
                        for h in range(JT_FD // 512):
                            nc.tensor.matmul(
                                L[:, h * 512 : (h + 1) * 512],
                                lhsT=lhs,
                                rhs=f2t_b[:, j0 + h * 512 : j0 + (h + 1) * 512],
                                start=True, stop=False, skip_group_check=True,
                            )
                    # inject -sq2/2 (shared ones_row weights)
                    for jt, j0, s_sb, L in pair:
                        for h in range(JT_FD // 512):
                            nc.tensor.matmul(
                                L[:, h * 512 : (h + 1) * 512],
                                lhsT=ones_row,
                                rhs=negsq2[:, j0 + h * 512 : j0 + (h + 1) * 512],
                                start=False, stop=True, skip_group_check=True,
                            )
                    # column sums of s for R (shared ones_col weights)
                    for jt, j0, s_sb, L in pair:
                        for h in range(JT_FD // 512):
                            nc.tensor.matmul(
                                cs_ps[:, h * 512 : (h + 1) * 512],
                                lhsT=ones_col,
                                rhs=s_sb[:, h * 512 : (h + 1) * 512],
                                start=(tix == 0), stop=(tix == n_tiles - 1),
                                skip_group_check=True,
                            )
                        tix += 1

                    for jt, j0, s_sb, L in pair:
                        e_sb = epool.tile([128, JT_FD], bf16, tag="e")
                        nc.scalar.activation(
                            out=e_sb, in_=L,
                            func=mybir.ActivationFunctionType.Exp,
                            scale=2.0, bias=0.0,
                            accum_out=zp[:, ic, jt : jt + 1],
                        )
                        se_sb = junk.tile([128, JT_FD], bf16, tag="se")
                        nc.vector.affine_mul_reduce(
                            out=se_sb, accum_out=tp[:, ic, jt : jt + 1],
                            in0=e_sb, in1=s_sb, scale=1.0, bias=0.0,
                        )

            # ------------- epilog: q = t/Z, column sums out ----------------
            zt = stats.tile([128, IC], f32)
            tt = stats.tile([128, IC], f32)
            nc.vector.reduce_sum(zt, zp, axis=mybir.AxisListType.X)
            nc.vector.reduce_sum(tt, tp, axis=mybir.AxisListType.X)
            zinv = stats.tile([128, IC], f32)
            nc.vector.reciprocal(zinv, zt)
            qv = stats.tile([128, IC], f32)
            nc.vector.tensor_mul(qv, tt, zinv)
            nc.sync.dma_start(out=q_out, in_=qv)
            cs_sb = stats.tile([1, JT_FD], f32)
            nc.vector.tensor_copy(cs_sb, cs_ps)
            nc.sync.dma_start(out=cs_out, in_=cs_sb)

    nc.compile()
    _CACHED["nc"] = nc
    return nc


def _run(feat1, feat2, simi, trace=False, **kwargs):
    from concourse import bass_utils

    nc = _build_module()

    feat1 = np.asarray(feat1, dtype=np.float32)
    feat2 = np.asarray(feat2, dtype=np.float32)
    simi = np.asarray(simi, dtype=np.int32)

    f1t_full = np.ascontiguousarray(feat1.T)  # [D, N]
    f2t_full = np.ascontiguousarray(feat2.T)  # [D, N]

    in_maps = []
    for c in range(N_CORES):
        in_maps.append(
            {
                "f1t": np.ascontiguousarray(f1t_full[:, c * NI : (c + 1) * NI]),
                "f2t": f2t_full,
                "s": np.ascontiguousarray(simi[c * NI : (c + 1) * NI, :]),
            }
        )

    res = bass_utils.run_bass_kernel_spmd(
        nc, in_maps, core_ids=list(range(N_CORES)), trace=trace, **kwargs
    )

    total_q = 0.0
    total_r = 0.0
    for c in range(N_CORES):
        total_q += float(np.sum(res.results[c]["q"], dtype=np.float64))
        total_r += float(np.sum(res.results[c]["cs"], dtype=np.float64))

    n2 = float(N) * float(N)
    loss = ((n2 - total_r) / 2.0 + total_q) / n2
    return np.float32(loss), res


def kernel(feat1: np.ndarray, feat2: np.ndarray, simi: np.ndarray) -> np.ndarray:
    loss, _ = _run(feat1, feat2, simi)
    return loss
